# revision 7
# baseline (speedup 1.0000x reference)
"""BinaryNet forward pass on 8 Trainium2 NeuronCores.

Strategy:
  - Data-parallel convs: 4 images/core, activations resident in SBUF in
    channel-major padded layout [C, H+2, W+2]. Binary convs run as 9-tap
    accumulating bf16 matmuls (+-1 exact, fp32 PSUM accumulation -> exact
    integer sums). conv0 (real input) runs in fp32 via host im2col (K=27).
  - BN+sign folded to per-channel thresholds: sign(BN(h)) == sign(h - t),
    t = mean - beta*sqrt(var+eps). Maxpool commutes with the monotone BN,
    so we sign first (1 ACT op from PSUM) then pool +-1 with tensor_max.
  - FC0 sharded over the contraction dim (9216 rows/core): AllToAll of the
    final conv activations (spatial-sharded), per-core partial matmul,
    AllReduce of (32,1024) partials. FC1/FC2 replicated (tiny).
  - Weights binarized + packed on host (the sharding hint replicates
    *binary* weights); BN thresholds folded on host.

SBUF pools alternate between the left/right allocation stacks so the
pipeline lifetimes (a_l dies after layer l+1) nest LIFO per side.
"""

import sys

for _p in ("/opt/trn_rl_repo",):
    if _p not in sys.path:
        sys.path.insert(0, _p)

from contextlib import ExitStack

import numpy as np
import ml_dtypes

from concourse import bacc
import concourse.tile as tile
import concourse.mybir as mybir
from concourse.alu_op_type import AluOpType
from concourse.bass_utils import run_bass_kernel_spmd
from concourse.masks import make_identity

F32 = mybir.dt.float32
BF16 = mybir.dt.bfloat16
N_CORES = 8
BN_EPS = 1e-3
DEBUG = False


# --------------------------------------------------------------------------
# device kernel
# --------------------------------------------------------------------------

def build_kernel(nimg=4):
    """Build the per-core Bass program. nimg = images per core."""
    nc = bacc.Bacc(num_devices=N_CORES)
    NB = nimg * N_CORES  # total batch

    # ---- inputs ----
    xpatch = nc.dram_tensor("xpatch", (32 * nimg, 9216), F32, kind="ExternalInput")
    wconv = nc.dram_tensor("wconv", (128, 279 * 128), BF16, kind="ExternalInput")
    w0pk = nc.dram_tensor("w0pk", (128, 128), F32, kind="ExternalInput")
    wfc0 = nc.dram_tensor("wfc0", (72, 128, 1024), BF16, kind="ExternalInput")
    wfc1 = nc.dram_tensor("wfc1", (8, 128, 1024), BF16, kind="ExternalInput")
    wfc2 = nc.dram_tensor("wfc2", (8, 128, 10), BF16, kind="ExternalInput")
    nthr = nc.dram_tensor("nthr", (128, 14), F32, kind="ExternalInput")
    t6bc = nc.dram_tensor("t6bc", (NB, 1024), F32, kind="ExternalInput")
    t7bc = nc.dram_tensor("t7bc", (NB, 1024), F32, kind="ExternalInput")
    s8bc = nc.dram_tensor("s8bc", (NB, 10), F32, kind="ExternalInput")
    b8bc = nc.dram_tensor("b8bc", (NB, 10), F32, kind="ExternalInput")

    out = nc.dram_tensor("out", (NB, 10), F32, kind="ExternalOutput")
    if DEBUG:
        d_a1 = nc.dram_tensor("d_a1", (128, 98, 98), BF16, kind="ExternalOutput")
        d_a6 = nc.dram_tensor("d_a6", (128, 12, 12), BF16, kind="ExternalOutput")
        d_A = nc.dram_tensor("d_A", (NB, 9216), BF16, kind="ExternalOutput")
        d_h6 = nc.dram_tensor("d_h6", (NB, 1024), F32, kind="ExternalOutput")

    # threshold column index within nthr for (layer, cob)
    TCOL = {(0, 0): 0, (1, 0): 1, (2, 0): 2, (2, 1): 3, (3, 0): 4, (3, 1): 5,
            (4, 0): 6, (4, 1): 7, (4, 2): 8, (4, 3): 9,
            (5, 0): 10, (5, 1): 11, (5, 2): 12, (5, 3): 13}
    # weight tile index base per conv layer within wconv (in 128-col tiles)
    WBASE = {1: 0, 2: 9, 3: 27, 4: 63, 5: 135}
    NCIB = {1: 1, 2: 1, 3: 2, 4: 2, 5: 4}
    NCOB = {1: 1, 2: 2, 3: 2, 4: 4, 5: 4}
    SIDE = {0: "left", 1: "right", 2: "left", 3: "right", 4: "left",
            5: "right", 6: "left"}  # generation -> SBUF side

    with tile.TileContext(nc) as tc:
        es = ExitStack()
        es_ps = ExitStack()
        ps = es_ps.enter_context(tc.tile_pool(name="ps", bufs=6, space="PSUM"))
        tmp = es.enter_context(tc.tile_pool(name="tmp", bufs=4))
        glob = es.enter_context(tc.tile_pool(name="glob", bufs=1))
        dram = es.enter_context(tc.tile_pool(name="dram", bufs=1, space="DRAM"))

        # ---- global small loads ----
        nthr_sb = glob.tile([128, 14], F32)
        nc.gpsimd.dma_start(nthr_sb[:], nthr[:])
        t6_sb = glob.tile([NB, 1024], F32)
        nc.gpsimd.dma_start(t6_sb[:], t6bc[:])
        t7_sb = glob.tile([NB, 1024], F32)
        nc.gpsimd.dma_start(t7_sb[:], t7bc[:])
        s8_sb = glob.tile([NB, 10], F32)
        nc.gpsimd.dma_start(s8_sb[:], s8bc[:])
        b8_sb = glob.tile([NB, 10], F32)
        nc.gpsimd.dma_start(b8_sb[:], b8bc[:])
        ident = glob.tile([128, 128], BF16)
        make_identity(nc, ident[:])
        w0_sb = glob.tile([128, 128], F32)
        nc.sync.dma_start(w0_sb[:], w0pk[:])

        def nbias(layer, cob):
            c = TCOL[(layer, cob)]
            return nthr_sb[:, c:c + 1]

        # ---- weight pools (opened one generation ahead, per-side stacks) ----
        es_w = {}
        wsb = {}

        def open_w(layer, side):
            es_w[layer] = ExitStack()
            p = es_w[layer].enter_context(
                tc.tile_pool(name=f"w{layer}", bufs=1, side=side))
            ntile = 9 * NCIB[layer] * NCOB[layer]
            t = p.tile([128, ntile * 128], BF16, name=f"w{layer}sb")
            b = WBASE[layer] * 128
            nc.sync.dma_start(t[:], wconv[:, b:b + ntile * 128])
            wsb[layer] = t

        def wtile(layer, tap, cib, cob):
            i = (tap * NCIB[layer] + cib) * NCOB[layer] + cob
            return wsb[layer][:, i * 128:(i + 1) * 128]

        # ================= gen 0 (left): w1, a1, patches =================
        open_w(1, SIDE[0])
        es_a1 = ExitStack()
        p_a1 = es_a1.enter_context(tc.tile_pool(name="a1", bufs=1, side=SIDE[0]))
        a1pad = []
        for img in range(nimg):
            t = p_a1.tile([128, 98, 98], BF16, name=f"a1_{img}")
            nc.vector.memset(t[:], 0.0)
            a1pad.append(t)
        es_pat = ExitStack()
        p_pat = es_pat.enter_context(tc.tile_pool(name="pat", bufs=2, side=SIDE[0]))

        # ---- L0: conv0 via im2col fp32 (patches streamed per image) ----
        for img in range(nimg):
            pat_t = p_pat.tile([32, 9216], F32, name="pat")
            nc.sync.dma_start(pat_t[:], xpatch[img * 32:(img + 1) * 32, :])
            for t24 in range(24):  # 4 output rows per tile
                cps = ps.tile([128, 4, 96], F32, name="cps")
                nc.tensor.matmul(
                    cps[:],
                    w0_sb[0:27, :],
                    pat_t[0:27, t24 * 384:(t24 + 1) * 384]
                    .rearrange("p (r c) -> p r c", r=4),
                    start=True, stop=True)
                nc.scalar.sign(a1pad[img][:, 1 + 4 * t24:5 + 4 * t24, 1:97],
                               cps[:], bias=nbias(0, 0))
        es_pat.close()

        # ================= gen 1 (right): w2, a2 =================
        open_w(2, SIDE[1])
        es_a2 = ExitStack()
        p_a2 = es_a2.enter_context(tc.tile_pool(name="a2", bufs=1, side=SIDE[1]))
        a2pad = []
        for img in range(nimg):
            t = p_a2.tile([128, 50, 50], BF16, name=f"a2_{img}")
            nc.vector.memset(t[:], 0.0)
            a2pad.append(t)

        # ---- L1: conv1 + pool -> a2pad ----
        for img in range(nimg):
            for t24 in range(24):
                cps = ps.tile([128, 4, 96], F32, name="cps")
                for tap in range(9):
                    ty, tx = divmod(tap, 3)
                    nc.tensor.matmul(
                        cps[:], wtile(1, tap, 0, 0),
                        a1pad[img][:, 4 * t24 + ty:4 * t24 + ty + 4, tx:tx + 96],
                        start=(tap == 0), stop=(tap == 8))
                sg = tmp.tile([128, 4, 96], BF16, name="sg")
                nc.scalar.sign(sg[:], cps[:], bias=nbias(1, 0))
                cm = tmp.tile([128, 4, 48], BF16, name="cm")
                nc.vector.tensor_max(cm[:], sg[:, :, 0::2], sg[:, :, 1::2])
                nc.vector.tensor_max(
                    a2pad[img][:, 1 + 2 * t24:3 + 2 * t24, 1:49],
                    cm[:, 0::2, :], cm[:, 1::2, :])
        if DEBUG:
            nc.sync.dma_start(d_a1[:], a1pad[0][:])
        es_a1.close()
        es_w[1].close()

        # ================= gen 2 (left): w3, a3 =================
        open_w(3, SIDE[2])
        es_a3 = ExitStack()
        p_a3 = es_a3.enter_context(tc.tile_pool(name="a3", bufs=1, side=SIDE[2]))
        a3pad = []
        for img in range(nimg):
            row = []
            for cb in range(2):
                t = p_a3.tile([128, 50, 50], BF16, name=f"a3_{img}_{cb}")
                nc.vector.memset(t[:], 0.0)
                row.append(t)
            a3pad.append(row)

        # ---- L2: conv2 -> a3pad (no pool) ----
        for img in range(nimg):
            for t6 in range(6):
                for cob in range(2):
                    cps = ps.tile([128, 8, 48], F32, name="cps")
                    for tap in range(9):
                        ty, tx = divmod(tap, 3)
                        nc.tensor.matmul(
                            cps[:], wtile(2, tap, 0, cob),
                            a2pad[img][:, 8 * t6 + ty:8 * t6 + ty + 8, tx:tx + 48],
                            start=(tap == 0), stop=(tap == 8))
                    nc.scalar.sign(a3pad[img][cob][:, 1 + 8 * t6:9 + 8 * t6, 1:49],
                                   cps[:], bias=nbias(2, cob))
        es_a2.close()
        es_w[2].close()

        # ================= gen 3 (right): w4, a4 =================
        open_w(4, SIDE[3])
        es_a4 = ExitStack()
        p_a4 = es_a4.enter_context(tc.tile_pool(name="a4", bufs=1, side=SIDE[3]))
        a4pad = []
        for img in range(nimg):
            row = []
            for cb in range(2):
                t = p_a4.tile([128, 26, 26], BF16, name=f"a4_{img}_{cb}")
                nc.vector.memset(t[:], 0.0)
                row.append(t)
            a4pad.append(row)

        # ---- L3: conv3 + pool -> a4pad ----
        for img in range(nimg):
            for t6 in range(6):
                for cob in range(2):
                    cps = ps.tile([128, 8, 48], F32, name="cps")
                    k = 0
                    for tap in range(9):
                        ty, tx = divmod(tap, 3)
                        for cib in range(2):
                            nc.tensor.matmul(
                                cps[:], wtile(3, tap, cib, cob),
                                a3pad[img][cib][:, 8 * t6 + ty:8 * t6 + ty + 8, tx:tx + 48],
                                start=(k == 0), stop=(k == 17))
                            k += 1
                    sg = tmp.tile([128, 8, 48], BF16, name="sg")
                    nc.scalar.sign(sg[:], cps[:], bias=nbias(3, cob))
                    cm = tmp.tile([128, 8, 24], BF16, name="cm")
                    nc.vector.tensor_max(cm[:], sg[:, :, 0::2], sg[:, :, 1::2])
                    nc.vector.tensor_max(
                        a4pad[img][cob][:, 1 + 4 * t6:5 + 4 * t6, 1:25],
                        cm[:, 0::2, :], cm[:, 1::2, :])
        es_a3.close()
        es_w[3].close()

        # ================= gen 4 (left): w5, a5 =================
        open_w(5, SIDE[4])
        es_a5 = ExitStack()
        p_a5 = es_a5.enter_context(tc.tile_pool(name="a5", bufs=1, side=SIDE[4]))
        a5pad = []
        for img in range(nimg):
            row = []
            for cb in range(4):
                t = p_a5.tile([128, 26, 26], BF16, name=f"a5_{img}_{cb}")
                nc.vector.memset(t[:], 0.0)
                row.append(t)
            a5pad.append(row)

        # ---- L4: conv4 -> a5pad (no pool) ----
        for img in range(nimg):
            for t2 in range(2):
                for cob in range(4):
                    cps = ps.tile([128, 12, 24], F32, name="cps")
                    k = 0
                    for tap in range(9):
                        ty, tx = divmod(tap, 3)
                        for cib in range(2):
                            nc.tensor.matmul(
                                cps[:], wtile(4, tap, cib, cob),
                                a4pad[img][cib][:, 12 * t2 + ty:12 * t2 + ty + 12, tx:tx + 24],
                                start=(k == 0), stop=(k == 17))
                            k += 1
                    nc.scalar.sign(a5pad[img][cob][:, 1 + 12 * t2:13 + 12 * t2, 1:25],
                                   cps[:], bias=nbias(4, cob))
        es_a4.close()
        es_w[4].close()

        # ================= gen 5 (right): wfc (w1/w2/w0-stream), a6 =========
        es_wfc = ExitStack()
        p_wfc = es_wfc.enter_context(tc.tile_pool(name="wfc", bufs=1, side=SIDE[5]))
        w1_sb = p_wfc.tile([128, 8, 1024], BF16)
        nc.sync.dma_start(w1_sb[:], wfc1[:].rearrange("q p j -> p q j"))
        w2_sb = p_wfc.tile([128, 8, 10], BF16)
        nc.sync.dma_start(w2_sb[:], wfc2[:].rearrange("q p j -> p q j"))
        p_w0s = es_wfc.enter_context(tc.tile_pool(name="w0s", bufs=40, side=SIDE[5]))

        es_a6 = ExitStack()
        p_a6 = es_a6.enter_context(tc.tile_pool(name="a6", bufs=1, side=SIDE[5]))
        a6 = []
        for img in range(nimg):
            row = []
            for cb in range(4):
                t = p_a6.tile([128, 12, 12], BF16, name=f"a6_{img}_{cb}")
                row.append(t)
            a6.append(row)

        # ---- L5: conv5 + pool -> a6 ----
        for img in range(nimg):
            for t2 in range(2):
                for cob in range(4):
                    cps = ps.tile([128, 12, 24], F32, name="cps")
                    k = 0
                    for tap in range(9):
                        ty, tx = divmod(tap, 3)
                        for cib in range(4):
                            nc.tensor.matmul(
                                cps[:], wtile(5, tap, cib, cob),
                                a5pad[img][cib][:, 12 * t2 + ty:12 * t2 + ty + 12, tx:tx + 24],
                                start=(k == 0), stop=(k == 35))
                            k += 1
                    sg = tmp.tile([128, 12, 24], BF16, name="sg")
                    nc.scalar.sign(sg[:], cps[:], bias=nbias(5, cob))
                    cm = tmp.tile([128, 12, 12], BF16, name="cm")
                    nc.vector.tensor_max(cm[:], sg[:, :, 0::2], sg[:, :, 1::2])
                    nc.vector.tensor_max(a6[img][cob][:, 6 * t2:6 * t2 + 6, :],
                                         cm[:, 0::2, :], cm[:, 1::2, :])
        es_a5.close()
        es_w[5].close()

        # ================= gen 6 (left): fc working set =================
        es_fc = ExitStack()
        p_fc = es_fc.enter_context(tc.tile_pool(name="fc", bufs=1, side=SIDE[6]))
        sT = []  # sT[img][half]: [72 s, 512 c]
        for img in range(nimg):
            halves = []
            for h in range(2):
                t = p_fc.tile([72, 512], BF16, name=f"sT_{img}_{h}")
                halves.append(t)
            sT.append(halves)

        # PE-transpose a6 [128c, 72s] -> [72s, 128c] (PSUM bf16), copy to sT
        es_ps.close()
        es_ptr = ExitStack()
        ps_tr = es_ptr.enter_context(tc.tile_pool(name="ps_tr", bufs=2, space="PSUM"))
        for img in range(nimg):
            for cb in range(4):
                for h in range(2):
                    ptr = ps_tr.tile([72, 128], BF16, name="ptr")
                    nc.tensor.transpose(
                        ptr[:],
                        a6[img][cb][:, 6 * h:6 * h + 6, :].rearrange("p a b -> p (a b)"),
                        ident[:])
                    nc.vector.tensor_copy(sT[img][h][:, cb * 128:(cb + 1) * 128],
                                          ptr[:])

        # ---- AllToAll exchange: spatial shard s in [18k, 18k+18) to core k ----
        send = dram.tile([N_CORES, nimg, 18, 512], BF16)
        recv = dram.tile([N_CORES * nimg, 9216], BF16)
        for k in range(N_CORES):
            h, r0 = divmod(18 * k, 72)
            for img in range(nimg):
                nc.sync.dma_start(send[k, img], sT[img][h][r0:r0 + 18, :])
        nc.gpsimd.collective_compute(
            "AllToAll", AluOpType.bypass,
            replica_groups=[list(range(N_CORES))],
            ins=[send.opt()], outs=[recv.opt()])

        A_loc = p_fc.tile([NB, 9216], BF16)
        nc.sync.dma_start(A_loc[:], recv[:])
        if DEBUG:
            nc.sync.dma_start(d_A[:], A_loc[:])
            nc.sync.dma_start(d_a6[:], a6[0][0][:])
        es_a6.close()

        # ---- FC0: partial over local 9216 contraction rows ----
        es_ptr.close()
        psfc = es_fc.enter_context(tc.tile_pool(name="psfc", bufs=1, space="PSUM"))
        p_aT = es_fc.enter_context(tc.tile_pool(name="aT", bufs=8, side=SIDE[6]))
        psA = psfc.tile([NB, 512], F32, name="psA")
        psB = psfc.tile([NB, 512], F32, name="psB")
        for q in range(72):
            aT = p_aT.tile([128, NB], BF16, name="aT")
            nc.sync.dma_start(aT[:], A_loc[:, q * 128:(q + 1) * 128], transpose=True)
            w0c = p_w0s.tile([128, 1024], BF16, name="w0c")
            nc.sync.dma_start(w0c[:], wfc0[q])
            nc.tensor.matmul(psA[:], aT[:], w0c[:, 0:512],
                             start=(q == 0), stop=(q == 71))
            nc.tensor.matmul(psB[:], aT[:], w0c[:, 512:1024],
                             start=(q == 0), stop=(q == 71))

        h6 = p_fc.tile([NB, 1024], F32)
        nc.vector.tensor_copy(h6[:, 0:512], psA[:])
        nc.vector.tensor_copy(h6[:, 512:1024], psB[:])
        arin = dram.tile([NB, 1024], F32)
        arout = dram.tile([NB, 1024], F32, addr_space="Shared")
        nc.sync.dma_start(arin[:], h6[:])
        nc.gpsimd.collective_compute(
            "AllReduce", AluOpType.add,
            replica_groups=[list(range(N_CORES))],
            ins=[arin.opt()], outs=[arout.opt()])
        h6r = p_fc.tile([NB, 1024], F32)
        nc.sync.dma_start(h6r[:], arout[:])
        if DEBUG:
            nc.sync.dma_start(d_h6[:], h6r[:])

        # threshold -> +-1 bf16
        ge6 = p_fc.tile([NB, 1024], F32)
        nc.vector.tensor_tensor(ge6[:], h6r[:], t6_sb[:], op=AluOpType.is_ge)
        a1fc = p_fc.tile([NB, 1024], BF16)
        nc.scalar.activation(a1fc[:], ge6[:], mybir.ActivationFunctionType.Copy,
                             bias=-1.0, scale=2.0)

        # ---- FC1 (replicated) ----
        psC = psfc.tile([NB, 512], F32, name="psC")
        psD = psfc.tile([NB, 512], F32, name="psD")
        for q in range(8):
            aT = p_aT.tile([128, NB], BF16, name="aT")
            nc.sync.dma_start(aT[:], a1fc[:, q * 128:(q + 1) * 128], transpose=True)
            nc.tensor.matmul(psC[:], aT[:], w1_sb[:, q, 0:512],
                             start=(q == 0), stop=(q == 7))
            nc.tensor.matmul(psD[:], aT[:], w1_sb[:, q, 512:1024],
                             start=(q == 0), stop=(q == 7))
        h7 = p_fc.tile([NB, 1024], F32)
        nc.vector.tensor_copy(h7[:, 0:512], psC[:])
        nc.vector.tensor_copy(h7[:, 512:1024], psD[:])
        ge7 = p_fc.tile([NB, 1024], F32)
        nc.vector.tensor_tensor(ge7[:], h7[:], t7_sb[:], op=AluOpType.is_ge)
        a2fc = p_fc.tile([NB, 1024], BF16)
        nc.scalar.activation(a2fc[:], ge7[:], mybir.ActivationFunctionType.Copy,
                             bias=-1.0, scale=2.0)

        # ---- FC2 + BN8 + softmax ----
        psE = psfc.tile([NB, 10], F32, name="psE")
        for q in range(8):
            aT = p_aT.tile([128, NB], BF16, name="aT")
            nc.sync.dma_start(aT[:], a2fc[:, q * 128:(q + 1) * 128], transpose=True)
            nc.tensor.matmul(psE[:], aT[:], w2_sb[:, q, :],
                             start=(q == 0), stop=(q == 7))
        z0 = p_fc.tile([NB, 10], F32)
        nc.vector.tensor_copy(z0[:], psE[:])
        z1 = p_fc.tile([NB, 10], F32)
        nc.vector.tensor_tensor(z1[:], z0[:], s8_sb[:], op=AluOpType.mult)
        z = p_fc.tile([NB, 10], F32)
        nc.vector.tensor_tensor(z[:], z1[:], b8_sb[:], op=AluOpType.add)
        zmax = p_fc.tile([NB, 1], F32)
        nc.vector.tensor_reduce(zmax[:], z[:], mybir.AxisListType.X, AluOpType.max)
        zc = p_fc.tile([NB, 10], F32)
        nc.vector.tensor_scalar(zc[:], z[:], zmax[:], None, op0=AluOpType.subtract)
        ez = p_fc.tile([NB, 10], F32)
        nc.scalar.activation(ez[:], zc[:], mybir.ActivationFunctionType.Exp)
        sez = p_fc.tile([NB, 1], F32)
        nc.vector.tensor_reduce(sez[:], ez[:], mybir.AxisListType.X, AluOpType.add)
        rez = p_fc.tile([NB, 1], F32)
        nc.vector.reciprocal(rez[:], sez[:])
        pr = p_fc.tile([NB, 10], F32)
        nc.vector.tensor_scalar(pr[:], ez[:], rez[:], None, op0=AluOpType.mult)
        nc.sync.dma_start(out[:], pr[:])

        es_fc.close()
        es_wfc.close()
        es.close()

    nc.finalize()
    return nc


# --------------------------------------------------------------------------
# host-side preparation
# --------------------------------------------------------------------------

def _sign_pm1(a):
    return np.where(np.asarray(a, np.float32) >= 0, np.float32(1.0), np.float32(-1.0))


def _thresholds(bn):
    beta, mean, var = [np.asarray(v, np.float64) for v in bn]
    return (mean - beta * np.sqrt(var + BN_EPS)).astype(np.float32)


def _nudge_even_int(t):
    """Sign(0)=0 on the ACT LUT; if a threshold lands exactly on an attainable
    (even-integer) conv sum, nudge it down one ulp so h==t yields +1."""
    t = t.copy()
    mask = (t == 2.0 * np.round(t / 2.0))
    t[mask] = np.nextafter(t[mask], -np.inf)
    return t


def prepare_inputs(x, conv_kernels, fc_kernels, bn_params, nimg=4):
    x = np.asarray(x, np.float32)
    B = x.shape[0]
    assert B == nimg * N_CORES

    # conv0 im2col, packed images into 32-row partition groups
    xpatches = []
    for c in range(N_CORES):
        pk = np.zeros((32 * nimg, 9216), np.float32)
        for i in range(nimg):
            im = x[c * nimg + i]
            xp = np.zeros((98, 98, 3), np.float32)
            xp[1:97, 1:97, :] = im
            for tap in range(9):
                ty, tx = divmod(tap, 3)
                blk = xp[ty:ty + 96, tx:tx + 96, :]  # [96,96,3]
                pk[32 * i + 3 * tap: 32 * i + 3 * tap + 3] = (
                    blk.transpose(2, 0, 1).reshape(3, 9216))
        xpatches.append(pk)

    # w0 packed: rows 32i + (3*tap+ci), replicated per image group, fp32 +-1
    w0 = _sign_pm1(conv_kernels[0]).reshape(9, 3, 128)  # [tap, ci, co]
    w0flat = w0.reshape(27, 128)
    w0pk = np.zeros((128, 128), np.float32)
    for i in range(4):
        w0pk[32 * i:32 * i + 27] = w0flat

    # conv weights 1..5 packed as [128, 279*128] bf16
    NCIB = {1: 1, 2: 1, 3: 2, 4: 2, 5: 4}
    NCOB = {1: 1, 2: 2, 3: 2, 4: 4, 5: 4}
    tiles = []
    for layer in range(1, 6):
        wb = _sign_pm1(conv_kernels[layer])  # [3,3,ci,co]
        for tap in range(9):
            ty, tx = divmod(tap, 3)
            for cib in range(NCIB[layer]):
                for cob in range(NCOB[layer]):
                    tiles.append(wb[ty, tx,
                                    cib * 128:(cib + 1) * 128,
                                    cob * 128:(cob + 1) * 128])
    wconv = np.concatenate(tiles, axis=1).astype(ml_dtypes.bfloat16)
    assert wconv.shape == (128, 279 * 128)

    # FC weights
    f0 = _sign_pm1(fc_kernels[0]).astype(ml_dtypes.bfloat16)  # (73728, 1024)
    wfc0 = [f0[9216 * k:9216 * (k + 1)].reshape(72, 128, 1024) for k in range(N_CORES)]
    wfc1 = _sign_pm1(fc_kernels[1]).astype(ml_dtypes.bfloat16).reshape(8, 128, 1024)
    wfc2 = _sign_pm1(fc_kernels[2]).astype(ml_dtypes.bfloat16).reshape(8, 128, 10)

    # conv thresholds, negated, packed [128, 14]
    t = [_thresholds(bn) for bn in bn_params]
    for l in range(1, 6):
        t[l] = _nudge_even_int(t[l])
    nthr = np.zeros((128, 14), np.float32)
    nthr[:, 0] = -t[0]
    nthr[:, 1] = -t[1]
    nthr[:, 2] = -t[2][0:128];  nthr[:, 3] = -t[2][128:256]
    nthr[:, 4] = -t[3][0:128];  nthr[:, 5] = -t[3][128:256]
    for b in range(4):
        nthr[:, 6 + b] = -t[4][128 * b:128 * (b + 1)]
        nthr[:, 10 + b] = -t[5][128 * b:128 * (b + 1)]

    t6bc = np.broadcast_to(t[6], (B, 1024)).astype(np.float32).copy()
    t7bc = np.broadcast_to(t[7], (B, 1024)).astype(np.float32).copy()
    beta8, mean8, var8 = [np.asarray(v, np.float64) for v in bn_params[8]]
    s8 = (1.0 / np.sqrt(var8 + BN_EPS))
    b8 = beta8 - mean8 * s8
    s8bc = np.broadcast_to(s8.astype(np.float32), (B, 10)).copy()
    b8bc = np.broadcast_to(b8.astype(np.float32), (B, 10)).copy()

    in_maps = []
    for c in range(N_CORES):
        in_maps.append(dict(
            xpatch=xpatches[c], wconv=wconv, w0pk=w0pk,
            wfc0=np.ascontiguousarray(wfc0[c]), wfc1=wfc1, wfc2=wfc2,
            nthr=nthr, t6bc=t6bc, t7bc=t7bc, s8bc=s8bc, b8bc=b8bc))
    return in_maps


# --------------------------------------------------------------------------
# entry point
# --------------------------------------------------------------------------

_CACHED = {}


def _get_nc(nimg=4):
    if nimg not in _CACHED:
        _CACHED[nimg] = build_kernel(nimg=nimg)
    return _CACHED[nimg]


def kernel(x, conv_kernels, fc_kernels, bn_params, _trace=False, _nimg=4):
    nc = _get_nc(_nimg)
    in_maps = prepare_inputs(x, conv_kernels, fc_kernels, bn_params, nimg=_nimg)
    res = run_bass_kernel_spmd(nc, in_maps, core_ids=list(range(N_CORES)),
                               trace=_trace)
    kernel.last_result = res
    return np.asarray(res.results[0]["out"], np.float32)


# revision 12
# speedup vs baseline: 1.1591x; 1.1591x over previous
"""BinaryNet forward pass on 8 Trainium2 NeuronCores.

Strategy:
  - Data-parallel convs: 4 images/core, activations resident in SBUF in
    channel-major padded layout [C, H+2, W+2]. Binary convs run as 9-tap
    accumulating bf16 matmuls (+-1 exact, fp32 PSUM accumulation -> exact
    integer sums). conv0 (real input) runs in fp32 via host im2col (K=27).
  - BN+sign folded to per-channel thresholds: sign(BN(h)) == sign(h - t),
    t = mean - beta*sqrt(var+eps). Maxpool commutes with the monotone BN,
    so we sign first (1 ACT op from PSUM) then pool +-1 with tensor_max.
  - FC0 sharded over the contraction dim (9216 rows/core): AllToAll of the
    final conv activations (spatial-sharded), per-core partial matmul,
    AllReduce of (32,1024) partials. FC1/FC2 replicated (tiny).
  - Weights binarized + packed on host (the sharding hint replicates
    *binary* weights); BN thresholds folded on host.

SBUF pools alternate between the left/right allocation stacks so the
pipeline lifetimes (a_l dies after layer l+1) nest LIFO per side.
"""

import sys

for _p in ("/opt/trn_rl_repo",):
    if _p not in sys.path:
        sys.path.insert(0, _p)

from contextlib import ExitStack

import numpy as np
import ml_dtypes

from concourse import bacc
import concourse.tile as tile
import concourse.mybir as mybir
from concourse.alu_op_type import AluOpType
from concourse.bass_utils import run_bass_kernel_spmd
from concourse.masks import make_identity

F32 = mybir.dt.float32
BF16 = mybir.dt.bfloat16
N_CORES = 8
BN_EPS = 1e-3
DEBUG = False


# --------------------------------------------------------------------------
# device kernel
# --------------------------------------------------------------------------

def build_kernel(nimg=4):
    """Build the per-core Bass program. nimg = images per core."""
    nc = bacc.Bacc(num_devices=N_CORES)
    NB = nimg * N_CORES  # total batch

    # ---- inputs ----
    xpatch = nc.dram_tensor("xpatch", (32 * nimg, 9216), F32, kind="ExternalInput")
    wconv = nc.dram_tensor("wconv", (128, 279 * 128), BF16, kind="ExternalInput")
    w0pk = nc.dram_tensor("w0pk", (128, 128), F32, kind="ExternalInput")
    wfc0 = nc.dram_tensor("wfc0", (72, 128, 1024), BF16, kind="ExternalInput")
    wfc1 = nc.dram_tensor("wfc1", (8, 128, 1024), BF16, kind="ExternalInput")
    wfc2 = nc.dram_tensor("wfc2", (8, 128, 10), BF16, kind="ExternalInput")
    nthr = nc.dram_tensor("nthr", (128, 14), F32, kind="ExternalInput")
    t6bc = nc.dram_tensor("t6bc", (NB, 1024), F32, kind="ExternalInput")
    t7bc = nc.dram_tensor("t7bc", (NB, 1024), F32, kind="ExternalInput")
    s8bc = nc.dram_tensor("s8bc", (NB, 10), F32, kind="ExternalInput")
    b8bc = nc.dram_tensor("b8bc", (NB, 10), F32, kind="ExternalInput")

    out = nc.dram_tensor("out", (NB, 10), F32, kind="ExternalOutput")
    if DEBUG:
        d_a1 = nc.dram_tensor("d_a1", (128, 98, 98), BF16, kind="ExternalOutput")
        d_a6 = nc.dram_tensor("d_a6", (128, 12, 12), BF16, kind="ExternalOutput")
        d_A = nc.dram_tensor("d_A", (NB, 9216), BF16, kind="ExternalOutput")
        d_h6 = nc.dram_tensor("d_h6", (NB, 1024), F32, kind="ExternalOutput")

    # threshold column index within nthr for (layer, cob)
    TCOL = {(0, 0): 0, (1, 0): 1, (2, 0): 2, (2, 1): 3, (3, 0): 4, (3, 1): 5,
            (4, 0): 6, (4, 1): 7, (4, 2): 8, (4, 3): 9,
            (5, 0): 10, (5, 1): 11, (5, 2): 12, (5, 3): 13}
    # weight tile index base per conv layer within wconv (in 128-col tiles)
    WBASE = {1: 0, 2: 9, 3: 27, 4: 63, 5: 135}
    NCIB = {1: 1, 2: 1, 3: 2, 4: 2, 5: 4}
    NCOB = {1: 1, 2: 2, 3: 2, 4: 4, 5: 4}
    SIDE = {0: "left", 1: "right", 2: "left", 3: "right", 4: "left",
            5: "right", 6: "left"}  # generation -> SBUF side

    with tile.TileContext(nc) as tc:
        es = ExitStack()
        es_ps = ExitStack()
        ps = es_ps.enter_context(tc.tile_pool(name="ps", bufs=6, space="PSUM"))
        tmp = es.enter_context(tc.tile_pool(name="tmp", bufs=4))
        glob = es.enter_context(tc.tile_pool(name="glob", bufs=1))
        dram = es.enter_context(tc.tile_pool(name="dram", bufs=1, space="DRAM"))

        # ---- global small loads ----
        nthr_sb = glob.tile([128, 14], F32)
        nc.gpsimd.dma_start(nthr_sb[:], nthr[:])
        t6_sb = glob.tile([NB, 1024], F32)
        nc.gpsimd.dma_start(t6_sb[:], t6bc[:])
        t7_sb = glob.tile([NB, 1024], F32)
        nc.gpsimd.dma_start(t7_sb[:], t7bc[:])
        s8_sb = glob.tile([NB, 10], F32)
        nc.gpsimd.dma_start(s8_sb[:], s8bc[:])
        b8_sb = glob.tile([NB, 10], F32)
        nc.gpsimd.dma_start(b8_sb[:], b8bc[:])
        ident = glob.tile([128, 128], BF16)
        make_identity(nc, ident[:])
        # HAM warm-up: ~5us of back-to-back matmuls so the PE clock is at
        # 2.4GHz before the real work starts
        for _ in range(48):
            wm = ps.tile([128, 128], F32, name="cps")
            nc.tensor.matmul(wm[:], ident[:], ident[:], start=True, stop=True)
        w0_sb = glob.tile([128, 128], F32)
        nc.sync.dma_start(w0_sb[:], w0pk[:])

        def nbias(layer, cob):
            c = TCOL[(layer, cob)]
            return nthr_sb[:, c:c + 1]

        # ---- weight pools (opened one generation ahead, per-side stacks) ----
        es_w = {}
        wsb = {}

        def open_w(layer, side):
            es_w[layer] = ExitStack()
            p = es_w[layer].enter_context(
                tc.tile_pool(name=f"w{layer}", bufs=1, side=side))
            ntile = 9 * NCIB[layer] * NCOB[layer]
            t = p.tile([128, ntile * 128], BF16, name=f"w{layer}sb")
            b = WBASE[layer] * 128
            nc.sync.dma_start(t[:], wconv[:, b:b + ntile * 128])
            wsb[layer] = t

        def wtile(layer, tap, cib, cob):
            i = (tap * NCIB[layer] + cib) * NCOB[layer] + cob
            return wsb[layer][:, i * 128:(i + 1) * 128]

        # ================= gen 0 (left): w1, a1, patches =================
        open_w(1, SIDE[0])
        es_a1 = ExitStack()
        p_a1 = es_a1.enter_context(tc.tile_pool(name="a1", bufs=1, side=SIDE[0]))
        a1pad = []
        for img in range(nimg):
            t = p_a1.tile([128, 98, 98], BF16, name=f"a1_{img}")
            nc.vector.memset(t[:], 0.0)
            a1pad.append(t)
        es_pat = ExitStack()
        p_pat = es_pat.enter_context(tc.tile_pool(name="pat", bufs=2, side=SIDE[0]))

        # ---- L0: conv0 via im2col fp32 (patches streamed per image) ----
        for img in range(nimg):
            pat_t = p_pat.tile([32, 9216], F32, name="pat")
            nc.sync.dma_start(pat_t[:], xpatch[img * 32:(img + 1) * 32, :])
            for t24 in range(24):  # 4 output rows per tile
                cps = ps.tile([128, 4, 96], F32, name="cps")
                nc.tensor.matmul(
                    cps[:],
                    w0_sb[0:27, :],
                    pat_t[0:27, t24 * 384:(t24 + 1) * 384]
                    .rearrange("p (r c) -> p r c", r=4),
                    start=True, stop=True)
                nc.scalar.sign(a1pad[img][:, 1 + 4 * t24:5 + 4 * t24, 1:97],
                               cps[:], bias=nbias(0, 0))
        es_pat.close()

        # ================= gen 1 (right): w2, a2 =================
        open_w(2, SIDE[1])
        es_a2 = ExitStack()
        p_a2 = es_a2.enter_context(tc.tile_pool(name="a2", bufs=1, side=SIDE[1]))
        a2pad = []
        for img in range(nimg):
            t = p_a2.tile([128, 50, 50], BF16, name=f"a2_{img}")
            nc.vector.memset(t[:], 0.0)
            a2pad.append(t)

        # ---- L1: conv1 + pool -> a2pad ----
        for img in range(nimg):
            for t24 in range(24):
                cps = ps.tile([128, 4, 96], F32, name="cps")
                for tap in range(9):
                    ty, tx = divmod(tap, 3)
                    nc.tensor.matmul(
                        cps[:], wtile(1, tap, 0, 0),
                        a1pad[img][:, 4 * t24 + ty:4 * t24 + ty + 4, tx:tx + 96],
                        start=(tap == 0), stop=(tap == 8))
                sg = tmp.tile([128, 4, 96], BF16, name="sg")
                nc.scalar.sign(sg[:], cps[:], bias=nbias(1, 0))
                cm = tmp.tile([128, 4, 48], BF16, name="cm")
                nc.vector.tensor_max(cm[:], sg[:, :, 0::2], sg[:, :, 1::2])
                nc.vector.tensor_max(
                    a2pad[img][:, 1 + 2 * t24:3 + 2 * t24, 1:49],
                    cm[:, 0::2, :], cm[:, 1::2, :])
        if DEBUG:
            nc.sync.dma_start(d_a1[:], a1pad[0][:])
        es_a1.close()
        es_w[1].close()

        # ================= gen 2 (left): w3, a3 =================
        open_w(3, SIDE[2])
        es_a3 = ExitStack()
        p_a3 = es_a3.enter_context(tc.tile_pool(name="a3", bufs=1, side=SIDE[2]))
        a3pad = []
        for img in range(nimg):
            row = []
            for cb in range(2):
                t = p_a3.tile([128, 50, 50], BF16, name=f"a3_{img}_{cb}")
                nc.vector.memset(t[:], 0.0)
                row.append(t)
            a3pad.append(row)

        # ---- L2: conv2 -> a3pad (no pool) ----
        for img in range(nimg):
            for t6 in range(6):
                for cob in range(2):
                    cps = ps.tile([128, 8, 48], F32, name="cps")
                    for tap in range(9):
                        ty, tx = divmod(tap, 3)
                        nc.tensor.matmul(
                            cps[:], wtile(2, tap, 0, cob),
                            a2pad[img][:, 8 * t6 + ty:8 * t6 + ty + 8, tx:tx + 48],
                            start=(tap == 0), stop=(tap == 8))
                    nc.scalar.sign(a3pad[img][cob][:, 1 + 8 * t6:9 + 8 * t6, 1:49],
                                   cps[:], bias=nbias(2, cob))
        es_a2.close()
        es_w[2].close()

        # ================= gen 3 (right): w4, a4 =================
        open_w(4, SIDE[3])
        es_a4 = ExitStack()
        p_a4 = es_a4.enter_context(tc.tile_pool(name="a4", bufs=1, side=SIDE[3]))
        a4pad = []
        for img in range(nimg):
            row = []
            for cb in range(2):
                t = p_a4.tile([128, 26, 26], BF16, name=f"a4_{img}_{cb}")
                nc.vector.memset(t[:], 0.0)
                row.append(t)
            a4pad.append(row)

        # ---- L3: conv3 + pool -> a4pad ----
        for img in range(nimg):
            for t6 in range(6):
                for cob in range(2):
                    cps = ps.tile([128, 8, 48], F32, name="cps")
                    k = 0
                    for tap in range(9):
                        ty, tx = divmod(tap, 3)
                        for cib in range(2):
                            nc.tensor.matmul(
                                cps[:], wtile(3, tap, cib, cob),
                                a3pad[img][cib][:, 8 * t6 + ty:8 * t6 + ty + 8, tx:tx + 48],
                                start=(k == 0), stop=(k == 17))
                            k += 1
                    sg = tmp.tile([128, 8, 48], BF16, name="sg")
                    nc.scalar.sign(sg[:], cps[:], bias=nbias(3, cob))
                    cm = tmp.tile([128, 8, 24], BF16, name="cm")
                    nc.vector.tensor_max(cm[:], sg[:, :, 0::2], sg[:, :, 1::2])
                    nc.vector.tensor_max(
                        a4pad[img][cob][:, 1 + 4 * t6:5 + 4 * t6, 1:25],
                        cm[:, 0::2, :], cm[:, 1::2, :])
        es_a3.close()
        es_w[3].close()

        # ================= gen 4 (left): w5, a5 =================
        open_w(5, SIDE[4])
        es_a5 = ExitStack()
        p_a5 = es_a5.enter_context(tc.tile_pool(name="a5", bufs=1, side=SIDE[4]))
        a5pad = []
        for img in range(nimg):
            row = []
            for cb in range(4):
                t = p_a5.tile([128, 26, 26], BF16, name=f"a5_{img}_{cb}")
                nc.vector.memset(t[:], 0.0)
                row.append(t)
            a5pad.append(row)

        # ---- L4: conv4 -> a5pad (no pool) ----
        for img in range(nimg):
            for t2 in range(2):
                for cob in range(4):
                    cps = ps.tile([128, 12, 24], F32, name="cps")
                    k = 0
                    for tap in range(9):
                        ty, tx = divmod(tap, 3)
                        for cib in range(2):
                            nc.tensor.matmul(
                                cps[:], wtile(4, tap, cib, cob),
                                a4pad[img][cib][:, 12 * t2 + ty:12 * t2 + ty + 12, tx:tx + 24],
                                start=(k == 0), stop=(k == 17))
                            k += 1
                    nc.scalar.sign(a5pad[img][cob][:, 1 + 12 * t2:13 + 12 * t2, 1:25],
                                   cps[:], bias=nbias(4, cob))
        es_a4.close()
        es_w[4].close()

        # ================= gen 5 (right): wfc (w1/w2/w0-stream), a6 =========
        es_wfc = ExitStack()
        p_wfc = es_wfc.enter_context(tc.tile_pool(name="wfc", bufs=1, side=SIDE[5]))
        w1_sb = p_wfc.tile([128, 8, 1024], BF16)
        nc.sync.dma_start(w1_sb[:], wfc1[:].rearrange("q p j -> p q j"))
        w2_sb = p_wfc.tile([128, 8, 10], BF16)
        nc.sync.dma_start(w2_sb[:], wfc2[:].rearrange("q p j -> p q j"))
        p_w0s = es_wfc.enter_context(tc.tile_pool(name="w0s", bufs=40, side=SIDE[5]))

        es_a6 = ExitStack()
        p_a6 = es_a6.enter_context(tc.tile_pool(name="a6", bufs=1, side=SIDE[5]))
        a6 = []
        for img in range(nimg):
            row = []
            for cb in range(4):
                t = p_a6.tile([128, 12, 12], BF16, name=f"a6_{img}_{cb}")
                row.append(t)
            a6.append(row)

        # ================= gen 6 (left): sT staging =================
        es_sT = ExitStack()
        p_sT = es_sT.enter_context(tc.tile_pool(name="sTp", bufs=1, side=SIDE[6]))
        sT = []  # sT[img][half]: [72 s, 512 c]
        for img in range(nimg):
            halves = []
            for h in range(2):
                t = p_sT.tile([72, 512], BF16, name=f"sT_{img}_{h}")
                halves.append(t)
            sT.append(halves)
        es_ptr = ExitStack()
        ps_tr = es_ptr.enter_context(tc.tile_pool(name="ps_tr", bufs=2, space="PSUM"))
        send = dram.tile([N_CORES, nimg, 18, 512], BF16)
        recv = dram.tile([N_CORES * nimg, 9216], BF16)

        # ---- L5: conv5 + pool -> a6; per-image sT transpose + send ----
        for img in range(nimg):
            for t2 in range(2):
                for cob in range(4):
                    cps = ps.tile([128, 12, 24], F32, name="cps")
                    k = 0
                    for tap in range(9):
                        ty, tx = divmod(tap, 3)
                        for cib in range(4):
                            nc.tensor.matmul(
                                cps[:], wtile(5, tap, cib, cob),
                                a5pad[img][cib][:, 12 * t2 + ty:12 * t2 + ty + 12, tx:tx + 24],
                                start=(k == 0), stop=(k == 35))
                            k += 1
                    sg = tmp.tile([128, 12, 24], BF16, name="sg")
                    nc.scalar.sign(sg[:], cps[:], bias=nbias(5, cob))
                    cm = tmp.tile([128, 12, 12], BF16, name="cm")
                    nc.vector.tensor_max(cm[:], sg[:, :, 0::2], sg[:, :, 1::2])
                    nc.vector.tensor_max(a6[img][cob][:, 6 * t2:6 * t2 + 6, :],
                                         cm[:, 0::2, :], cm[:, 1::2, :])
            # PE-transpose this image's a6 [128c, 72s] -> [72s, 128c], then send
            for cb in range(4):
                for h in range(2):
                    ptr = ps_tr.tile([72, 128], BF16, name="ptr")
                    nc.tensor.transpose(
                        ptr[:],
                        a6[img][cb][:, 6 * h:6 * h + 6, :].rearrange("p a b -> p (a b)"),
                        ident[:])
                    nc.vector.tensor_copy(sT[img][h][:, cb * 128:(cb + 1) * 128],
                                          ptr[:])
            for k in range(N_CORES):
                h, r0 = divmod(18 * k, 72)
                nc.sync.dma_start(send[k, img], sT[img][h][r0:r0 + 18, :])
        es_sT.close()
        es_a5.close()
        es_w[5].close()
        es_fc = ExitStack()
        p_fc = es_fc.enter_context(tc.tile_pool(name="fc", bufs=1, side=SIDE[6]))

        # ---- AllToAll exchange: spatial shard s in [18k, 18k+18) to core k ----
        nc.gpsimd.collective_compute(
            "AllToAll", AluOpType.bypass,
            replica_groups=[list(range(N_CORES))],
            ins=[send.opt()], outs=[recv.opt()])

        A_loc = p_fc.tile([NB, 9216], BF16)
        nc.sync.dma_start(A_loc[:], recv[:])
        if DEBUG:
            nc.sync.dma_start(d_A[:], A_loc[:])
            nc.sync.dma_start(d_a6[:], a6[0][0][:])
        es_a6.close()

        # ---- FC0: partial over local 9216 contraction rows ----
        es_ptr.close()
        es_ps.close()
        psfc = es_fc.enter_context(tc.tile_pool(name="psfc", bufs=1, space="PSUM"))
        p_aT = es_fc.enter_context(tc.tile_pool(name="aT", bufs=72, side=SIDE[6]))

        def pe_transpose_chunk(src_ap, name):
            # [32, 128] sbuf -> [128, 32] sbuf via PE transpose (PSUM bounce)
            pq = psfc.tile([128, NB], BF16, name="ptq", bufs=2)
            nc.tensor.transpose(pq[:], src_ap, ident[0:NB, 0:NB])
            t = p_aT.tile([128, NB], BF16, name=name)
            nc.vector.tensor_copy(t[:], pq[:])
            return t

        aTs = [pe_transpose_chunk(A_loc[:, q * 128:(q + 1) * 128], "aT")
               for q in range(72)]
        psA = psfc.tile([NB, 512], F32, name="psA")
        psB = psfc.tile([NB, 512], F32, name="psB")
        for q in range(72):
            w0c = p_w0s.tile([128, 1024], BF16, name="w0c")
            nc.sync.dma_start(w0c[:], wfc0[q])
            nc.tensor.matmul(psA[:], aTs[q][:], w0c[:, 0:512],
                             start=(q == 0), stop=(q == 71))
            nc.tensor.matmul(psB[:], aTs[q][:], w0c[:, 512:1024],
                             start=(q == 0), stop=(q == 71))

        h6 = p_fc.tile([NB, 1024], F32)
        nc.vector.tensor_copy(h6[:, 0:512], psA[:])
        nc.vector.tensor_copy(h6[:, 512:1024], psB[:])
        arin = dram.tile([NB, 1024], F32)
        arout = dram.tile([NB, 1024], F32, addr_space="Shared")
        nc.sync.dma_start(arin[:], h6[:])
        nc.gpsimd.collective_compute(
            "AllReduce", AluOpType.add,
            replica_groups=[list(range(N_CORES))],
            ins=[arin.opt()], outs=[arout.opt()])
        h6r = p_fc.tile([NB, 1024], F32)
        nc.sync.dma_start(h6r[:], arout[:])
        if DEBUG:
            nc.sync.dma_start(d_h6[:], h6r[:])

        # threshold -> +-1 bf16
        ge6 = p_fc.tile([NB, 1024], F32)
        nc.vector.tensor_tensor(ge6[:], h6r[:], t6_sb[:], op=AluOpType.is_ge)
        a1fc = p_fc.tile([NB, 1024], BF16)
        nc.scalar.activation(a1fc[:], ge6[:], mybir.ActivationFunctionType.Copy,
                             bias=-1.0, scale=2.0)

        # ---- FC1 (replicated) ----
        psC = psfc.tile([NB, 512], F32, name="psC")
        psD = psfc.tile([NB, 512], F32, name="psD")
        a1Ts = [pe_transpose_chunk(a1fc[:, q * 128:(q + 1) * 128], "aT")
                for q in range(8)]
        for q in range(8):
            nc.tensor.matmul(psC[:], a1Ts[q][:], w1_sb[:, q, 0:512],
                             start=(q == 0), stop=(q == 7))
            nc.tensor.matmul(psD[:], a1Ts[q][:], w1_sb[:, q, 512:1024],
                             start=(q == 0), stop=(q == 7))
        h7 = p_fc.tile([NB, 1024], F32)
        nc.vector.tensor_copy(h7[:, 0:512], psC[:])
        nc.vector.tensor_copy(h7[:, 512:1024], psD[:])
        ge7 = p_fc.tile([NB, 1024], F32)
        nc.vector.tensor_tensor(ge7[:], h7[:], t7_sb[:], op=AluOpType.is_ge)
        a2fc = p_fc.tile([NB, 1024], BF16)
        nc.scalar.activation(a2fc[:], ge7[:], mybir.ActivationFunctionType.Copy,
                             bias=-1.0, scale=2.0)

        # ---- FC2 + BN8 + softmax ----
        psE = psfc.tile([NB, 10], F32, name="psE")
        a2Ts = [pe_transpose_chunk(a2fc[:, q * 128:(q + 1) * 128], "aT")
                for q in range(8)]
        for q in range(8):
            nc.tensor.matmul(psE[:], a2Ts[q][:], w2_sb[:, q, :],
                             start=(q == 0), stop=(q == 7))
        z0 = p_fc.tile([NB, 10], F32)
        nc.vector.tensor_copy(z0[:], psE[:])
        z1 = p_fc.tile([NB, 10], F32)
        nc.vector.tensor_tensor(z1[:], z0[:], s8_sb[:], op=AluOpType.mult)
        z = p_fc.tile([NB, 10], F32)
        nc.vector.tensor_tensor(z[:], z1[:], b8_sb[:], op=AluOpType.add)
        zmax = p_fc.tile([NB, 1], F32)
        nc.vector.tensor_reduce(zmax[:], z[:], mybir.AxisListType.X, AluOpType.max)
        zc = p_fc.tile([NB, 10], F32)
        nc.vector.tensor_scalar(zc[:], z[:], zmax[:], None, op0=AluOpType.subtract)
        ez = p_fc.tile([NB, 10], F32)
        nc.scalar.activation(ez[:], zc[:], mybir.ActivationFunctionType.Exp)
        sez = p_fc.tile([NB, 1], F32)
        nc.vector.tensor_reduce(sez[:], ez[:], mybir.AxisListType.X, AluOpType.add)
        rez = p_fc.tile([NB, 1], F32)
        nc.vector.reciprocal(rez[:], sez[:])
        pr = p_fc.tile([NB, 10], F32)
        nc.vector.tensor_scalar(pr[:], ez[:], rez[:], None, op0=AluOpType.mult)
        nc.sync.dma_start(out[:], pr[:])

        es_fc.close()
        es_wfc.close()
        es.close()

    nc.finalize()
    return nc


# --------------------------------------------------------------------------
# host-side preparation
# --------------------------------------------------------------------------

def _sign_pm1(a):
    return np.where(np.asarray(a, np.float32) >= 0, np.float32(1.0), np.float32(-1.0))


def _thresholds(bn):
    beta, mean, var = [np.asarray(v, np.float64) for v in bn]
    return (mean - beta * np.sqrt(var + BN_EPS)).astype(np.float32)


def _nudge_even_int(t):
    """Sign(0)=0 on the ACT LUT; if a threshold lands exactly on an attainable
    (even-integer) conv sum, nudge it down one ulp so h==t yields +1."""
    t = t.copy()
    mask = (t == 2.0 * np.round(t / 2.0))
    t[mask] = np.nextafter(t[mask], -np.inf)
    return t


def prepare_inputs(x, conv_kernels, fc_kernels, bn_params, nimg=4):
    x = np.asarray(x, np.float32)
    B = x.shape[0]
    assert B == nimg * N_CORES

    # conv0 im2col, packed images into 32-row partition groups
    xpatches = []
    for c in range(N_CORES):
        pk = np.zeros((32 * nimg, 9216), np.float32)
        for i in range(nimg):
            im = x[c * nimg + i]
            xp = np.zeros((98, 98, 3), np.float32)
            xp[1:97, 1:97, :] = im
            for tap in range(9):
                ty, tx = divmod(tap, 3)
                blk = xp[ty:ty + 96, tx:tx + 96, :]  # [96,96,3]
                pk[32 * i + 3 * tap: 32 * i + 3 * tap + 3] = (
                    blk.transpose(2, 0, 1).reshape(3, 9216))
        xpatches.append(pk)

    # w0 packed: rows 32i + (3*tap+ci), replicated per image group, fp32 +-1
    w0 = _sign_pm1(conv_kernels[0]).reshape(9, 3, 128)  # [tap, ci, co]
    w0flat = w0.reshape(27, 128)
    w0pk = np.zeros((128, 128), np.float32)
    for i in range(4):
        w0pk[32 * i:32 * i + 27] = w0flat

    # conv weights 1..5 packed as [128, 279*128] bf16
    NCIB = {1: 1, 2: 1, 3: 2, 4: 2, 5: 4}
    NCOB = {1: 1, 2: 2, 3: 2, 4: 4, 5: 4}
    tiles = []
    for layer in range(1, 6):
        wb = _sign_pm1(conv_kernels[layer])  # [3,3,ci,co]
        for tap in range(9):
            ty, tx = divmod(tap, 3)
            for cib in range(NCIB[layer]):
                for cob in range(NCOB[layer]):
                    tiles.append(wb[ty, tx,
                                    cib * 128:(cib + 1) * 128,
                                    cob * 128:(cob + 1) * 128])
    wconv = np.concatenate(tiles, axis=1).astype(ml_dtypes.bfloat16)
    assert wconv.shape == (128, 279 * 128)

    # FC weights
    f0 = _sign_pm1(fc_kernels[0]).astype(ml_dtypes.bfloat16)  # (73728, 1024)
    wfc0 = [f0[9216 * k:9216 * (k + 1)].reshape(72, 128, 1024) for k in range(N_CORES)]
    wfc1 = _sign_pm1(fc_kernels[1]).astype(ml_dtypes.bfloat16).reshape(8, 128, 1024)
    wfc2 = _sign_pm1(fc_kernels[2]).astype(ml_dtypes.bfloat16).reshape(8, 128, 10)

    # conv thresholds, negated, packed [128, 14]
    t = [_thresholds(bn) for bn in bn_params]
    for l in range(1, 6):
        t[l] = _nudge_even_int(t[l])
    nthr = np.zeros((128, 14), np.float32)
    nthr[:, 0] = -t[0]
    nthr[:, 1] = -t[1]
    nthr[:, 2] = -t[2][0:128];  nthr[:, 3] = -t[2][128:256]
    nthr[:, 4] = -t[3][0:128];  nthr[:, 5] = -t[3][128:256]
    for b in range(4):
        nthr[:, 6 + b] = -t[4][128 * b:128 * (b + 1)]
        nthr[:, 10 + b] = -t[5][128 * b:128 * (b + 1)]

    t6bc = np.broadcast_to(t[6], (B, 1024)).astype(np.float32).copy()
    t7bc = np.broadcast_to(t[7], (B, 1024)).astype(np.float32).copy()
    beta8, mean8, var8 = [np.asarray(v, np.float64) for v in bn_params[8]]
    s8 = (1.0 / np.sqrt(var8 + BN_EPS))
    b8 = beta8 - mean8 * s8
    s8bc = np.broadcast_to(s8.astype(np.float32), (B, 10)).copy()
    b8bc = np.broadcast_to(b8.astype(np.float32), (B, 10)).copy()

    in_maps = []
    for c in range(N_CORES):
        in_maps.append(dict(
            xpatch=xpatches[c], wconv=wconv, w0pk=w0pk,
            wfc0=np.ascontiguousarray(wfc0[c]), wfc1=wfc1, wfc2=wfc2,
            nthr=nthr, t6bc=t6bc, t7bc=t7bc, s8bc=s8bc, b8bc=b8bc))
    return in_maps


# --------------------------------------------------------------------------
# entry point
# --------------------------------------------------------------------------

_CACHED = {}


def _get_nc(nimg=4):
    if nimg not in _CACHED:
        _CACHED[nimg] = build_kernel(nimg=nimg)
    return _CACHED[nimg]


def kernel(x, conv_kernels, fc_kernels, bn_params, _trace=False, _nimg=4):
    nc = _get_nc(_nimg)
    in_maps = prepare_inputs(x, conv_kernels, fc_kernels, bn_params, nimg=_nimg)
    res = run_bass_kernel_spmd(nc, in_maps, core_ids=list(range(N_CORES)),
                               trace=_trace)
    kernel.last_result = res
    return np.asarray(res.results[0]["out"], np.float32)
